# revision 12
# baseline (speedup 1.0000x reference)
"""MoE (top-2 routing, 8 experts, SwiGLU FFN) on 8 Trainium2 NeuronCores.

Strategy (expert-parallel, host-mediated all-to-all):
  - Host computes the router (softmax + top-2 + renormalize) in fp32,
    exactly mirroring the reference.
  - Tokens are gathered per expert (the "all-to-all") on the host, padded to a
    common capacity C, transposed to [H, C] and cast to bf16.
  - Core e runs the SwiGLU FFN of expert e over its token batch entirely from
    SBUF-resident bf16 weights:  y = (silu(x@Wg) * (x@Wu)) @ Wd, scaled by the
    renormalized router weight per token (applied on-chip in fp32).
  - Host combines: each token's output is the sum of its two expert rows.
  - The tiny aux loss is computed on host in fp32 (exact reference formula).

The device kernel keeps activations feature-major ([feat, token]) through the
up-projection so all weights are consumed in their natural layouts, and swaps
the stationary operand for the down-projection so the output comes out
token-major ([token, H]) and the per-token scale lands on the partition axis.
"""

import numpy as np
import ml_dtypes

B, S, H, I, E = 4, 2048, 1024, 2048, 8
TOP_K = 2
AUX_COEF = 1e-3
N_CORES = 8
P = 128          # SBUF partitions
TBLK = 512       # token block (matmul free dim)
KH = H // P      # 8 contraction chunks for up-proj
NI = I // P      # 16 I tiles
HH = H // TBLK   # 2 output column halves

_PROGRAM_CACHE: dict = {}


# ----------------------------------------------------------------------------
# Host-side routing (exact fp32 mirror of the reference router)
# ----------------------------------------------------------------------------

def _route(xf: np.ndarray, gate_w: np.ndarray):
    logits = xf @ gate_w                                   # [N, E] fp32
    m = logits.max(axis=-1, keepdims=True)
    ex = np.exp(logits - m, dtype=np.float32)
    rw = ex / ex.sum(axis=-1, keepdims=True)               # softmax fp32
    top_i = np.argsort(-rw, axis=-1, kind="stable")[:, :TOP_K]
    top_w = np.take_along_axis(rw, top_i, axis=1)          # unnormalized
    top_wn = top_w / top_w.sum(axis=-1, keepdims=True)     # renormalized
    return rw, top_i, top_w, top_wn


def _aux_loss(rw, top_i, top_w):
    N = rw.shape[0]
    flat_i = top_i.reshape(-1)
    load = np.bincount(flat_i, weights=top_w.reshape(-1).astype(np.float64),
                       minlength=E).astype(np.float32)
    counts = np.bincount(flat_i, minlength=E).astype(np.float32)
    P_expert = load / np.float32(N)
    P_token_expert = counts / np.float32(N * TOP_K)
    return np.float32((P_expert * P_token_expert).sum() * E * AUX_COEF)


# ----------------------------------------------------------------------------
# Device program (built once per capacity C, cached)
# ----------------------------------------------------------------------------

def _block_sizes(C: int):
    """Split C into token blocks: multiples of TBLK plus one 128-multiple tail."""
    blocks = [TBLK] * (C // TBLK)
    rem = C % TBLK
    if rem:
        blocks.append(rem)
    return blocks


def _build_program(C: int, reps: int = 1, down_i_outer: bool = False,
                   shared_psum: bool = False, body_mult: int = 1, skew: bool = True,
                   ldw_pair: bool = False, ldw_pair_down: bool = False):
    from contextlib import ExitStack
    import concourse.tile as tile
    from concourse import bacc, mybir

    assert C % P == 0
    NT = C // P
    bf16 = mybir.dt.bfloat16
    f32 = mybir.dt.float32
    AF = mybir.ActivationFunctionType

    nc = bacc.Bacc("TRN2", target_bir_lowering=False, debug=False,
                   enable_asserts=True, num_devices=N_CORES)

    xt_ap = nc.dram_tensor("xt", [KH, P, C], bf16, kind="ExternalInput").ap()
    wg_ap = nc.dram_tensor("wg", [KH, P, NI, P], bf16, kind="ExternalInput").ap()
    wu_ap = nc.dram_tensor("wu", [KH, P, NI, P], bf16, kind="ExternalInput").ap()
    wd_ap = nc.dram_tensor("wd", [NI, P, H], bf16, kind="ExternalInput").ap()
    sc_ap = nc.dram_tensor("sc", [P, NT], f32, kind="ExternalInput").ap()
    y_ap = nc.dram_tensor("y", [C, H], f32, kind="ExternalOutput").ap()

    with tile.TileContext(nc) as tc:
        with ExitStack() as ctx:
            wpool = ctx.enter_context(tc.tile_pool(name="weights", bufs=1))
            xpool = ctx.enter_context(tc.tile_pool(name="xb", bufs=2))
            gpool = ctx.enter_context(tc.tile_pool(name="g", bufs=2))
            spool = ctx.enter_context(tc.tile_pool(name="s1", bufs=2))
            ypool = ctx.enter_context(tc.tile_pool(name="yo", bufs=3))
            ppool = ctx.enter_context(tc.tile_pool(name="psum", bufs=2, space="PSUM"))

            # Persistent SBUF-resident weights (bf16): 12.6 MB total.
            wg_sb = [wpool.tile([P, NI, P], bf16, tag=f"wg{k}", name=f"wg_sb{k}") for k in range(KH)]
            wu_sb = [wpool.tile([P, NI, P], bf16, tag=f"wu{k}", name=f"wu_sb{k}") for k in range(KH)]
            wd_sb = [wpool.tile([P, H], bf16, tag=f"wd{i}", name=f"wd_sb{i}") for i in range(NI)]
            sc_sb = wpool.tile([P, NT], f32, tag="sc")
            # Chunked in i-consumption order: the first up-proj group needs
            # only i-chunk 0 of every k-tile (~2MB), not all 17MB of weights.
            ICH = 4
            for i0 in range(0, NI, ICH):
                for k in range(KH):
                    nc.sync.dma_start(wg_sb[k][:, i0:i0 + ICH, :],
                                      wg_ap[k][:, i0:i0 + ICH, :])
                    nc.sync.dma_start(wu_sb[k][:, i0:i0 + ICH, :],
                                      wu_ap[k][:, i0:i0 + ICH, :])
            nc.sync.dma_start(sc_sb[:], sc_ap[:])

            def body():
                blocks = _block_sizes(C)
                starts = [sum(blocks[:j]) for j in range(len(blocks))]
                wd_loaded = [False]

                def up_phase(b):
                    T = blocks[b]
                    tok0 = starts[b]
                    xb = xpool.tile([P, KH, T], bf16, tag="xb", name="xb")
                    for k in range(KH):
                        nc.sync.dma_start(xb[:, k, :], xt_ap[k][:, tok0:tok0 + T])

                    g = gpool.tile([P, NI, T], bf16, tag="g", name="g")
                    ptag = "ps" if shared_psum else "p1"
                    ptag2 = "ps" if shared_psum else "p2"
                    pbufs = 8 if shared_psum else 2
                    if ldw_pair:
                        Th = T // 2
                        for i in range(NI):
                            p1h = [ppool.tile([P, Th], f32, tag=f"p1{h}", bufs=1,
                                              name=f"p1{h}") for h in range(2)]
                            for k in range(KH):
                                for h in range(2):
                                    nc.tensor.matmul(
                                        p1h[h][:], wg_sb[k][:, i, :],
                                        xb[:, k, h * Th:(h + 1) * Th],
                                        start=(k == 0), stop=(k == KH - 1))
                            p2h = [ppool.tile([P, Th], f32, tag=f"p2{h}", bufs=1,
                                              name=f"p2{h}") for h in range(2)]
                            for k in range(KH):
                                for h in range(2):
                                    nc.tensor.matmul(
                                        p2h[h][:], wu_sb[k][:, i, :],
                                        xb[:, k, h * Th:(h + 1) * Th],
                                        start=(k == 0), stop=(k == KH - 1))
                            for h in range(2):
                                s1 = spool.tile([P, Th], f32, tag=f"s1{h}", name="s1")
                                nc.scalar.activation(s1[:], p1h[h][:], AF.Silu)
                                nc.vector.tensor_mul(
                                    out=g[:, i, h * Th:(h + 1) * Th],
                                    in0=s1[:], in1=p2h[h][:])
                    else:
                        for i in range(NI):
                            p1 = ppool.tile([P, T], f32, tag=ptag, bufs=pbufs, name="p1")
                            for k in range(KH):
                                nc.tensor.matmul(p1[:], wg_sb[k][:, i, :], xb[:, k, :],
                                                 start=(k == 0), stop=(k == KH - 1))
                            p2 = ppool.tile([P, T], f32, tag=ptag2, bufs=pbufs, name="p2")
                            for k in range(KH):
                                nc.tensor.matmul(p2[:], wu_sb[k][:, i, :], xb[:, k, :],
                                                 start=(k == 0), stop=(k == KH - 1))
                            s1 = spool.tile([P, T], f32, tag="s1", name="s1")
                            nc.scalar.activation(s1[:], p1[:], AF.Silu)
                            nc.vector.tensor_mul(out=g[:, i, :], in0=s1[:], in1=p2[:])

                    if not wd_loaded[0]:
                        # Emitted after the first block's up-proj so the x/Wg/Wu
                        # DMAs win the queues early; Wd is only needed ~80us in.
                        for i in range(NI):
                            nc.sync.dma_start(wd_sb[i][:], wd_ap[i])
                        wd_loaded[0] = True
                    return g

                def down_phase(b, g):
                    T = blocks[b]
                    tok0 = starts[b]
                    nt_off = tok0 // P
                    if down_i_outer:
                        nst = T // P
                        pys = [ppool.tile([P, TBLK], f32, tag="ps" if shared_psum else "py",
                                          bufs=8 if shared_psum else 2, name=f"py{j}")
                               for j in range(nst * HH)]
                        for i in range(NI):
                            for st in range(nst):
                                for hh in range(HH):
                                    nc.tensor.matmul(
                                        pys[st * HH + hh][:], g[:, i, st * P:(st + 1) * P],
                                        wd_sb[i][:, hh * TBLK:(hh + 1) * TBLK],
                                        start=(i == 0), stop=(i == NI - 1))
                        for st in range(nst):
                            for hh in range(HH):
                                yo = ypool.tile([P, TBLK], f32, tag="yo", name="yo")
                                nc.scalar.activation(
                                    yo[:], pys[st * HH + hh][:], AF.Copy,
                                    scale=sc_sb[:, nt_off + st:nt_off + st + 1])
                                nc.sync.dma_start(
                                    y_ap[tok0 + st * P:tok0 + (st + 1) * P,
                                         hh * TBLK:(hh + 1) * TBLK],
                                    yo[:])
                    elif ldw_pair or ldw_pair_down:
                        for st in range(T // P):
                            pyh = [ppool.tile([P, TBLK], f32, tag=f"py{h}", bufs=1,
                                              name=f"py{h}") for h in range(HH)]
                            for i in range(NI):
                                gsl = g[:, i, st * P:(st + 1) * P]
                                for hh in range(HH):
                                    nc.tensor.matmul(
                                        pyh[hh][:], gsl,
                                        wd_sb[i][:, hh * TBLK:(hh + 1) * TBLK],
                                        start=(i == 0), stop=(i == NI - 1))
                            for hh in range(HH):
                                yo = ypool.tile([P, TBLK], f32, tag="yo", name="yo")
                                nc.scalar.activation(
                                    yo[:], pyh[hh][:], AF.Copy,
                                    scale=sc_sb[:, nt_off + st:nt_off + st + 1])
                                nc.sync.dma_start(
                                    y_ap[tok0 + st * P:tok0 + (st + 1) * P,
                                         hh * TBLK:(hh + 1) * TBLK],
                                    yo[:])
                    else:
                        for st in range(T // P):
                            for hh in range(HH):
                                py = ppool.tile([P, TBLK], f32, tag="py", name="py")
                                for i in range(NI):
                                    nc.tensor.matmul(
                                        py[:], g[:, i, st * P:(st + 1) * P],
                                        wd_sb[i][:, hh * TBLK:(hh + 1) * TBLK],
                                        start=(i == 0), stop=(i == NI - 1))
                                yo = ypool.tile([P, TBLK], f32, tag="yo", name="yo")
                                nc.scalar.activation(
                                    yo[:], py[:], AF.Copy,
                                    scale=sc_sb[:, nt_off + st:nt_off + st + 1])
                                nc.sync.dma_start(
                                    y_ap[tok0 + st * P:tok0 + (st + 1) * P,
                                         hh * TBLK:(hh + 1) * TBLK],
                                    yo[:])

                if skew:
                    gs = {}
                    gs[0] = up_phase(0)
                    for b in range(1, len(blocks)):
                        gs[b] = up_phase(b)
                        down_phase(b - 1, gs.pop(b - 1))
                    down_phase(len(blocks) - 1, gs.pop(len(blocks) - 1))
                else:
                    for b in range(len(blocks)):
                        g = up_phase(b)
                        down_phase(b, g)

            if reps == 1:
                body()
            elif reps < 0:  # static python unroll (straight-line, no For_i)
                for _ in range(-reps):
                    body()
            else:
                with tc.For_i(0, reps, 1):
                    for _ in range(body_mult):
                        body()

    nc.compile()
    return nc


def _get_program(C: int, reps: int = 1, **opts):
    key = (C, reps, tuple(sorted(opts.items())))
    if key not in _PROGRAM_CACHE:
        _PROGRAM_CACHE[key] = _build_program(C, reps, **opts)
    return _PROGRAM_CACHE[key]


# ----------------------------------------------------------------------------
# Host-side shard / unshard + entry point
# ----------------------------------------------------------------------------

def _choose_capacity(counts):
    """Smallest 128-multiple capacity whose total overflow (token-assignments
    beyond capacity, computed on host instead) stays tiny."""
    cmax = int(counts.max())
    full = max(-(-cmax // P) * P, P)
    for C in range(max(-(-int(counts.min()) // P) * P, P), full, P):
        if int(np.maximum(counts - C, 0).sum()) <= 64:
            return C
    return full


def _prepare_in_maps(xf, w_gate, w_up, w_down, top_i, top_wn):
    N = xf.shape[0]
    flat_e = top_i.reshape(-1)
    order = np.argsort(flat_e, kind="stable")
    counts = np.bincount(flat_e, minlength=E)
    starts = np.concatenate([[0], np.cumsum(counts)])[:E]
    C = _choose_capacity(counts)
    NT = C // P

    # slot of each assignment within its expert's batch (>= C -> host overflow)
    slot_flat = np.empty(N * TOP_K, np.int64)
    slot_flat[order] = np.arange(N * TOP_K) - np.repeat(starts, counts)

    w_flat = top_wn.reshape(-1)
    bf16 = ml_dtypes.bfloat16
    in_maps = []
    for e in range(E):
        ndev = min(int(counts[e]), C)
        seg = order[starts[e]:starts[e] + ndev]
        toks = seg // TOP_K
        xe = np.zeros((C, H), np.float32)
        xe[:ndev] = xf[toks]
        xt = np.ascontiguousarray(xe.T).astype(bf16).reshape(KH, P, C)
        sc = np.zeros(C, np.float32)
        sc[:ndev] = w_flat[seg]
        sc = np.ascontiguousarray(sc.reshape(NT, P).T)
        wg = w_gate[e].astype(bf16).reshape(KH, P, NI, P)
        wu = w_up[e].astype(bf16).reshape(KH, P, NI, P)
        wd = w_down[e].astype(bf16).reshape(NI, P, H)
        in_maps.append({"xt": xt, "wg": wg, "wu": wu, "wd": wd, "sc": sc})
    return in_maps, C, slot_flat


def _combine(outs, top_i, slot_flat, C, N, xf=None, top_wn=None,
             w_gate=None, w_up=None, w_down=None):
    # outs: [E, C, H] fp32, already scaled by renormalized router weights
    flat = np.concatenate([outs.reshape(E * C, H),
                           np.zeros((1, H), np.float32)])  # sink row for overflow
    e0 = top_i[:, 0].astype(np.int64)
    e1 = top_i[:, 1].astype(np.int64)
    s0 = slot_flat[0::TOP_K].copy()
    s1 = slot_flat[1::TOP_K].copy()
    of0 = s0 >= C
    of1 = s1 >= C
    idx0 = np.where(of0, E * C, e0 * C + s0)
    idx1 = np.where(of1, E * C, e1 * C + s1)
    y = flat[idx0] + flat[idx1]
    # host-side fp32 epilogue for the few assignments beyond device capacity
    for k, of in ((0, of0), (1, of1)):
        for t in np.flatnonzero(of):
            e = int(top_i[t, k])
            h1 = xf[t] @ w_gate[e]
            h2 = xf[t] @ w_up[e]
            ge = (h1 / (1.0 + np.exp(-h1))) * h2
            y[t] += np.float32(top_wn[t, k]) * (ge @ w_down[e])
    return y


def kernel(x, gate_w, w_gate, w_up, w_down):
    from concourse.bass_utils import run_bass_kernel_spmd

    x = np.asarray(x, np.float32)
    gate_w = np.asarray(gate_w, np.float32)
    w_gate = np.asarray(w_gate, np.float32)
    w_up = np.asarray(w_up, np.float32)
    w_down = np.asarray(w_down, np.float32)

    Bx, Sx, Hx = x.shape
    N = Bx * Sx
    xf = x.reshape(N, Hx)

    rw, top_i, top_w, top_wn = _route(xf, gate_w)
    in_maps, C, slot_flat = _prepare_in_maps(
        xf, w_gate, w_up, w_down, top_i, top_wn)

    nc = _get_program(C)
    res = run_bass_kernel_spmd(nc, in_maps, core_ids=list(range(N_CORES)),
                               trace=False)
    outs = np.stack([res.results[c]["y"] for c in range(N_CORES)])

    y = _combine(outs, top_i, slot_flat, C, N, xf=xf, top_wn=top_wn,
                 w_gate=w_gate, w_up=w_up, w_down=w_down).reshape(Bx, Sx, Hx)
    aux = _aux_loss(rw, top_i, top_w)
    return y, aux


# revision 13
# speedup vs baseline: 1.0541x; 1.0541x over previous
"""MoE (top-2 routing, 8 experts, SwiGLU FFN) on 8 Trainium2 NeuronCores.

Strategy (expert-parallel, host-mediated all-to-all):
  - Host computes the router (softmax + top-2 + renormalize) in fp32,
    exactly mirroring the reference.
  - Tokens are gathered per expert (the "all-to-all") on the host, padded to a
    common capacity C, transposed to [H, C] and cast to bf16.
  - Core e runs the SwiGLU FFN of expert e over its token batch entirely from
    SBUF-resident bf16 weights:  y = (silu(x@Wg) * (x@Wu)) @ Wd, scaled by the
    renormalized router weight per token (applied on-chip in fp32).
  - Host combines: each token's output is the sum of its two expert rows.
  - The tiny aux loss is computed on host in fp32 (exact reference formula).

The device kernel keeps activations feature-major ([feat, token]) through the
up-projection so all weights are consumed in their natural layouts, and swaps
the stationary operand for the down-projection so the output comes out
token-major ([token, H]) and the per-token scale lands on the partition axis.
"""

import numpy as np
import ml_dtypes

B, S, H, I, E = 4, 2048, 1024, 2048, 8
TOP_K = 2
AUX_COEF = 1e-3
N_CORES = 8
P = 128          # SBUF partitions
TBLK = 512       # token block (matmul free dim)
KH = H // P      # 8 contraction chunks for up-proj
NI = I // P      # 16 I tiles
HH = H // TBLK   # 2 output column halves

_PROGRAM_CACHE: dict = {}


# ----------------------------------------------------------------------------
# Host-side routing (exact fp32 mirror of the reference router)
# ----------------------------------------------------------------------------

def _route(xf: np.ndarray, gate_w: np.ndarray):
    logits = xf @ gate_w                                   # [N, E] fp32
    m = logits.max(axis=-1, keepdims=True)
    ex = np.exp(logits - m, dtype=np.float32)
    rw = ex / ex.sum(axis=-1, keepdims=True)               # softmax fp32
    top_i = np.argsort(-rw, axis=-1, kind="stable")[:, :TOP_K]
    top_w = np.take_along_axis(rw, top_i, axis=1)          # unnormalized
    top_wn = top_w / top_w.sum(axis=-1, keepdims=True)     # renormalized
    return rw, top_i, top_w, top_wn


def _aux_loss(rw, top_i, top_w):
    N = rw.shape[0]
    flat_i = top_i.reshape(-1)
    load = np.bincount(flat_i, weights=top_w.reshape(-1).astype(np.float64),
                       minlength=E).astype(np.float32)
    counts = np.bincount(flat_i, minlength=E).astype(np.float32)
    P_expert = load / np.float32(N)
    P_token_expert = counts / np.float32(N * TOP_K)
    return np.float32((P_expert * P_token_expert).sum() * E * AUX_COEF)


# ----------------------------------------------------------------------------
# Device program (built once per capacity C, cached)
# ----------------------------------------------------------------------------

def _block_sizes(C: int):
    """Split C into token blocks: multiples of TBLK plus one 128-multiple tail."""
    blocks = [TBLK] * (C // TBLK)
    rem = C % TBLK
    if rem:
        blocks.append(rem)
    return blocks


def _build_program(C: int, reps: int = 1, down_i_outer: bool = False,
                   shared_psum: bool = False, body_mult: int = 1, skew: bool = True,
                   ldw_pair: bool = False, ldw_pair_down: bool = False,
                   skew_depth: int = 1):
    from contextlib import ExitStack
    import concourse.tile as tile
    from concourse import bacc, mybir

    assert C % P == 0
    NT = C // P
    bf16 = mybir.dt.bfloat16
    f32 = mybir.dt.float32
    AF = mybir.ActivationFunctionType

    nc = bacc.Bacc("TRN2", target_bir_lowering=False, debug=False,
                   enable_asserts=True, num_devices=N_CORES)

    xt_ap = nc.dram_tensor("xt", [KH, P, C], bf16, kind="ExternalInput").ap()
    wg_ap = nc.dram_tensor("wg", [KH, P, NI, P], bf16, kind="ExternalInput").ap()
    wu_ap = nc.dram_tensor("wu", [KH, P, NI, P], bf16, kind="ExternalInput").ap()
    wd_ap = nc.dram_tensor("wd", [NI, P, H], bf16, kind="ExternalInput").ap()
    sc_ap = nc.dram_tensor("sc", [P, NT], f32, kind="ExternalInput").ap()
    y_ap = nc.dram_tensor("y", [C, H], f32, kind="ExternalOutput").ap()

    with tile.TileContext(nc) as tc:
        with ExitStack() as ctx:
            wpool = ctx.enter_context(tc.tile_pool(name="weights", bufs=1))
            xpool = ctx.enter_context(tc.tile_pool(name="xb", bufs=1 + skew_depth))
            gpool = ctx.enter_context(tc.tile_pool(name="g", bufs=1 + skew_depth))
            spool = ctx.enter_context(tc.tile_pool(name="s1", bufs=2))
            ypool = ctx.enter_context(tc.tile_pool(name="yo", bufs=3))
            ppool = ctx.enter_context(tc.tile_pool(name="psum", bufs=2, space="PSUM"))

            # Persistent SBUF-resident weights (bf16): 12.6 MB total.
            wg_sb = [wpool.tile([P, NI, P], bf16, tag=f"wg{k}", name=f"wg_sb{k}") for k in range(KH)]
            wu_sb = [wpool.tile([P, NI, P], bf16, tag=f"wu{k}", name=f"wu_sb{k}") for k in range(KH)]
            wd_sb = [wpool.tile([P, H], bf16, tag=f"wd{i}", name=f"wd_sb{i}") for i in range(NI)]
            sc_sb = wpool.tile([P, NT], f32, tag="sc")
            # Chunked in i-consumption order: the first up-proj group needs
            # only i-chunk 0 of every k-tile (~2MB), not all 17MB of weights.
            ICH = 4
            for i0 in range(0, NI, ICH):
                for k in range(KH):
                    nc.sync.dma_start(wg_sb[k][:, i0:i0 + ICH, :],
                                      wg_ap[k][:, i0:i0 + ICH, :])
                    nc.sync.dma_start(wu_sb[k][:, i0:i0 + ICH, :],
                                      wu_ap[k][:, i0:i0 + ICH, :])
            nc.sync.dma_start(sc_sb[:], sc_ap[:])

            def body():
                blocks = _block_sizes(C)
                starts = [sum(blocks[:j]) for j in range(len(blocks))]
                wd_loaded = [False]

                def up_phase(b):
                    T = blocks[b]
                    tok0 = starts[b]
                    xb = xpool.tile([P, KH, T], bf16, tag="xb", name="xb")
                    for k in range(KH):
                        nc.sync.dma_start(xb[:, k, :], xt_ap[k][:, tok0:tok0 + T])

                    g = gpool.tile([P, NI, T], bf16, tag="g", name="g")
                    ptag = "ps" if shared_psum else "p1"
                    ptag2 = "ps" if shared_psum else "p2"
                    pbufs = 8 if shared_psum else 2
                    if ldw_pair:
                        Th = T // 2
                        for i in range(NI):
                            p1h = [ppool.tile([P, Th], f32, tag=f"p1{h}", bufs=1,
                                              name=f"p1{h}") for h in range(2)]
                            for k in range(KH):
                                for h in range(2):
                                    nc.tensor.matmul(
                                        p1h[h][:], wg_sb[k][:, i, :],
                                        xb[:, k, h * Th:(h + 1) * Th],
                                        start=(k == 0), stop=(k == KH - 1))
                            p2h = [ppool.tile([P, Th], f32, tag=f"p2{h}", bufs=1,
                                              name=f"p2{h}") for h in range(2)]
                            for k in range(KH):
                                for h in range(2):
                                    nc.tensor.matmul(
                                        p2h[h][:], wu_sb[k][:, i, :],
                                        xb[:, k, h * Th:(h + 1) * Th],
                                        start=(k == 0), stop=(k == KH - 1))
                            for h in range(2):
                                s1 = spool.tile([P, Th], f32, tag=f"s1{h}", name="s1")
                                nc.scalar.activation(s1[:], p1h[h][:], AF.Silu)
                                nc.vector.tensor_mul(
                                    out=g[:, i, h * Th:(h + 1) * Th],
                                    in0=s1[:], in1=p2h[h][:])
                    else:
                        for i in range(NI):
                            p1 = ppool.tile([P, T], f32, tag=ptag, bufs=pbufs, name="p1")
                            for k in range(KH):
                                nc.tensor.matmul(p1[:], wg_sb[k][:, i, :], xb[:, k, :],
                                                 start=(k == 0), stop=(k == KH - 1))
                            p2 = ppool.tile([P, T], f32, tag=ptag2, bufs=pbufs, name="p2")
                            for k in range(KH):
                                nc.tensor.matmul(p2[:], wu_sb[k][:, i, :], xb[:, k, :],
                                                 start=(k == 0), stop=(k == KH - 1))
                            s1 = spool.tile([P, T], f32, tag="s1", name="s1")
                            nc.scalar.activation(s1[:], p1[:], AF.Silu)
                            nc.vector.tensor_mul(out=g[:, i, :], in0=s1[:], in1=p2[:])

                    if not wd_loaded[0]:
                        # Emitted after the first block's up-proj so the x/Wg/Wu
                        # DMAs win the queues early; Wd is only needed ~80us in.
                        for i in range(NI):
                            nc.sync.dma_start(wd_sb[i][:], wd_ap[i])
                        wd_loaded[0] = True
                    return g

                def down_phase(b, g):
                    T = blocks[b]
                    tok0 = starts[b]
                    nt_off = tok0 // P
                    if down_i_outer:
                        nst = T // P
                        pys = [ppool.tile([P, TBLK], f32, tag="ps" if shared_psum else "py",
                                          bufs=8 if shared_psum else 2, name=f"py{j}")
                               for j in range(nst * HH)]
                        for i in range(NI):
                            for st in range(nst):
                                for hh in range(HH):
                                    nc.tensor.matmul(
                                        pys[st * HH + hh][:], g[:, i, st * P:(st + 1) * P],
                                        wd_sb[i][:, hh * TBLK:(hh + 1) * TBLK],
                                        start=(i == 0), stop=(i == NI - 1))
                        for st in range(nst):
                            for hh in range(HH):
                                yo = ypool.tile([P, TBLK], f32, tag="yo", name="yo")
                                nc.scalar.activation(
                                    yo[:], pys[st * HH + hh][:], AF.Copy,
                                    scale=sc_sb[:, nt_off + st:nt_off + st + 1])
                                nc.sync.dma_start(
                                    y_ap[tok0 + st * P:tok0 + (st + 1) * P,
                                         hh * TBLK:(hh + 1) * TBLK],
                                    yo[:])
                    elif ldw_pair or ldw_pair_down:
                        for st in range(T // P):
                            pyh = [ppool.tile([P, TBLK], f32, tag=f"py{h}", bufs=1,
                                              name=f"py{h}") for h in range(HH)]
                            for i in range(NI):
                                gsl = g[:, i, st * P:(st + 1) * P]
                                for hh in range(HH):
                                    nc.tensor.matmul(
                                        pyh[hh][:], gsl,
                                        wd_sb[i][:, hh * TBLK:(hh + 1) * TBLK],
                                        start=(i == 0), stop=(i == NI - 1))
                            for hh in range(HH):
                                yo = ypool.tile([P, TBLK], f32, tag="yo", name="yo")
                                nc.scalar.activation(
                                    yo[:], pyh[hh][:], AF.Copy,
                                    scale=sc_sb[:, nt_off + st:nt_off + st + 1])
                                nc.sync.dma_start(
                                    y_ap[tok0 + st * P:tok0 + (st + 1) * P,
                                         hh * TBLK:(hh + 1) * TBLK],
                                    yo[:])
                    else:
                        for st in range(T // P):
                            for hh in range(HH):
                                py = ppool.tile([P, TBLK], f32, tag="py", name="py")
                                for i in range(NI):
                                    nc.tensor.matmul(
                                        py[:], g[:, i, st * P:(st + 1) * P],
                                        wd_sb[i][:, hh * TBLK:(hh + 1) * TBLK],
                                        start=(i == 0), stop=(i == NI - 1))
                                yo = ypool.tile([P, TBLK], f32, tag="yo", name="yo")
                                nc.scalar.activation(
                                    yo[:], py[:], AF.Copy,
                                    scale=sc_sb[:, nt_off + st:nt_off + st + 1])
                                nc.sync.dma_start(
                                    y_ap[tok0 + st * P:tok0 + (st + 1) * P,
                                         hh * TBLK:(hh + 1) * TBLK],
                                    yo[:])

                if skew:
                    D = min(skew_depth, len(blocks) - 1)
                    gs = {}
                    for b in range(D):
                        gs[b] = up_phase(b)
                    for b in range(D, len(blocks)):
                        gs[b] = up_phase(b)
                        down_phase(b - D, gs.pop(b - D))
                    for b in range(len(blocks) - D, len(blocks)):
                        down_phase(b, gs.pop(b))
                else:
                    for b in range(len(blocks)):
                        g = up_phase(b)
                        down_phase(b, g)

            if reps == 1:
                body()
            elif reps < 0:  # static python unroll (straight-line, no For_i)
                for _ in range(-reps):
                    body()
            else:
                with tc.For_i(0, reps, 1):
                    for _ in range(body_mult):
                        body()

    nc.compile()
    return nc


def _get_program(C: int, reps: int = 1, **opts):
    key = (C, reps, tuple(sorted(opts.items())))
    if key not in _PROGRAM_CACHE:
        _PROGRAM_CACHE[key] = _build_program(C, reps, **opts)
    return _PROGRAM_CACHE[key]


# ----------------------------------------------------------------------------
# Host-side shard / unshard + entry point
# ----------------------------------------------------------------------------

def _choose_capacity(counts):
    """Smallest 128-multiple capacity whose total overflow (token-assignments
    beyond capacity, computed on host instead) stays tiny."""
    cmax = int(counts.max())
    full = max(-(-cmax // P) * P, P)
    for C in range(max(-(-int(counts.min()) // P) * P, P), full, P):
        if int(np.maximum(counts - C, 0).sum()) <= 64:
            return C
    return full


def _prepare_in_maps(xf, w_gate, w_up, w_down, top_i, top_wn):
    N = xf.shape[0]
    flat_e = top_i.reshape(-1)
    order = np.argsort(flat_e, kind="stable")
    counts = np.bincount(flat_e, minlength=E)
    starts = np.concatenate([[0], np.cumsum(counts)])[:E]
    C = _choose_capacity(counts)
    NT = C // P

    # slot of each assignment within its expert's batch (>= C -> host overflow)
    slot_flat = np.empty(N * TOP_K, np.int64)
    slot_flat[order] = np.arange(N * TOP_K) - np.repeat(starts, counts)

    w_flat = top_wn.reshape(-1)
    bf16 = ml_dtypes.bfloat16
    in_maps = []
    for e in range(E):
        ndev = min(int(counts[e]), C)
        seg = order[starts[e]:starts[e] + ndev]
        toks = seg // TOP_K
        xe = np.zeros((C, H), np.float32)
        xe[:ndev] = xf[toks]
        xt = np.ascontiguousarray(xe.T).astype(bf16).reshape(KH, P, C)
        sc = np.zeros(C, np.float32)
        sc[:ndev] = w_flat[seg]
        sc = np.ascontiguousarray(sc.reshape(NT, P).T)
        wg = w_gate[e].astype(bf16).reshape(KH, P, NI, P)
        wu = w_up[e].astype(bf16).reshape(KH, P, NI, P)
        wd = w_down[e].astype(bf16).reshape(NI, P, H)
        in_maps.append({"xt": xt, "wg": wg, "wu": wu, "wd": wd, "sc": sc})
    return in_maps, C, slot_flat


def _combine(outs, top_i, slot_flat, C, N, xf=None, top_wn=None,
             w_gate=None, w_up=None, w_down=None):
    # outs: [E, C, H] fp32, already scaled by renormalized router weights
    flat = np.concatenate([outs.reshape(E * C, H),
                           np.zeros((1, H), np.float32)])  # sink row for overflow
    e0 = top_i[:, 0].astype(np.int64)
    e1 = top_i[:, 1].astype(np.int64)
    s0 = slot_flat[0::TOP_K].copy()
    s1 = slot_flat[1::TOP_K].copy()
    of0 = s0 >= C
    of1 = s1 >= C
    idx0 = np.where(of0, E * C, e0 * C + s0)
    idx1 = np.where(of1, E * C, e1 * C + s1)
    y = flat[idx0] + flat[idx1]
    # host-side fp32 epilogue for the few assignments beyond device capacity
    for k, of in ((0, of0), (1, of1)):
        for t in np.flatnonzero(of):
            e = int(top_i[t, k])
            h1 = xf[t] @ w_gate[e]
            h2 = xf[t] @ w_up[e]
            ge = (h1 / (1.0 + np.exp(-h1))) * h2
            y[t] += np.float32(top_wn[t, k]) * (ge @ w_down[e])
    return y


def kernel(x, gate_w, w_gate, w_up, w_down):
    from concourse.bass_utils import run_bass_kernel_spmd

    x = np.asarray(x, np.float32)
    gate_w = np.asarray(gate_w, np.float32)
    w_gate = np.asarray(w_gate, np.float32)
    w_up = np.asarray(w_up, np.float32)
    w_down = np.asarray(w_down, np.float32)

    Bx, Sx, Hx = x.shape
    N = Bx * Sx
    xf = x.reshape(N, Hx)

    rw, top_i, top_w, top_wn = _route(xf, gate_w)
    in_maps, C, slot_flat = _prepare_in_maps(
        xf, w_gate, w_up, w_down, top_i, top_wn)

    nc = _get_program(C)
    res = run_bass_kernel_spmd(nc, in_maps, core_ids=list(range(N_CORES)),
                               trace=False)
    outs = np.stack([res.results[c]["y"] for c in range(N_CORES)])

    y = _combine(outs, top_i, slot_flat, C, N, xf=xf, top_wn=top_wn,
                 w_gate=w_gate, w_up=w_up, w_down=w_down).reshape(Bx, Sx, Hx)
    aux = _aux_loss(rw, top_i, top_w)
    return y, aux
